# revision 35
# baseline (speedup 1.0000x reference)
"""Trainium2 Bass kernel for nn_BatchProgramCC (tree-CNN + BiGRU program-pair
classifier). Self-contained: hardcodes shapes/sharding; builds+runs an 8-core
SPMD Bass program via run_bass_kernel_spmd.

Sharding: data-parallel over B (8 programs/core); embedding table (bf16) + all
weights replicated. Per core, per side: batched indirect-DMA gather of 16384
bf16 emb rows (8 calls x 2048 rows), DRAM-staged DMA-transpose to X=[E, tok],
fixed-topology subtree-sum (15 vector adds, before W_c by linearity), W_c
matmul + ACT relu-copy + vector tree-max -> enc, bulk n-gate input matmuls,
then a 128-step bidirectional GRU with two independent per-side dependency
chains (r/z input matmuls accumulated into per-step PSUM; elementwise spread
over Vector/GpSimd; activations on Scalar), time max-pool, fc + softmax.
"""
import os
import numpy as np
import ml_dtypes

# ---- problem constants (hardcoded per contract) ----
B, S, K = 64, 128, 16
MAX_DEPTH = 5
V, E, H = 50000, 128, 100
NCORES = 8
BL = B // NCORES            # programs per core = 8
NT = BL * S * K             # tokens per core per side = 16384
NTREE = BL * S              # trees per core = 1024
GCALLS = NT // 128          # 128 indirect calls per side (1 row/partition)

# fixed binary-tree topology (matches reference._tree_structure)
_LOCAL_PARENT = np.array([0] + [(i - 1) // 2 for i in range(1, K)], dtype=np.int64)
_LOCAL_LEVEL = np.floor(np.log2(np.arange(K) + 1)).astype(np.int64)
# child-sum edge schedule, bottom-up: (parent, child) pairs in dependency order
_EDGES = [(7, 15),
          (3, 7), (3, 8), (4, 9), (4, 10), (5, 11), (5, 12), (6, 13), (6, 14),
          (1, 3), (1, 4), (2, 5), (2, 6),
          (0, 1), (0, 2)]
_SUBSIZE = np.ones(K, np.int64)
for _k in range(K - 1, 0, -1):
    _SUBSIZE[(_k - 1) // 2] += _SUBSIZE[_k]


def _np_reference(tokens1, tokens2, parent, level, emb, W_c, b_c,
                  gru_wih_f, gru_whh_f, gru_bih_f, gru_bhh_f,
                  gru_wih_b, gru_whh_b, gru_bih_b, gru_bhh_b, fc_w, fc_b):
    """numpy fallback (used only if inputs are not the fixed topology /
    zero-bias case this kernel specializes for)."""
    def sigmoid(x):
        return 1.0 / (1.0 + np.exp(-x))

    def gru_dir(x, w_ih, w_hh, b_ih, b_hh):
        b, s, e = x.shape
        h = np.zeros((b, w_hh.shape[1]), np.float32)
        ys = np.empty((b, s, w_hh.shape[1]), np.float32)
        for t in range(s):
            gi = x[:, t] @ w_ih.T + b_ih
            gh = h @ w_hh.T + b_hh
            ir, iz, inn = np.split(gi, 3, axis=1)
            hr, hz, hn = np.split(gh, 3, axis=1)
            r = sigmoid(ir + hr)
            z = sigmoid(iz + hz)
            n = np.tanh(inn + r * hn)
            h = (1.0 - z) * n + z * h
            ys[:, t] = h
        return ys

    def encode(tokens):
        h = emb[tokens] @ W_c.T + b_c
        for d in range(MAX_DEPTH - 1, 0, -1):
            contrib = np.where((level == d)[:, None], h, 0.0)
            np.add.at(h, parent, contrib)
        enc = np.maximum(h.reshape(B, S, K, E).max(axis=2), 0.0)
        fwd = gru_dir(enc, gru_wih_f, gru_whh_f, gru_bih_f, gru_bhh_f)
        bwd = gru_dir(enc[:, ::-1], gru_wih_b, gru_whh_b, gru_bih_b, gru_bhh_b)[:, ::-1]
        return np.concatenate([fwd, bwd], axis=-1).max(axis=1)

    lvec = encode(tokens1)
    rvec = encode(tokens2)
    y = np.concatenate([lvec, rvec], axis=1) @ fc_w.T + fc_b
    y = y - y.max(axis=1, keepdims=True)
    ey = np.exp(y)
    return (ey / ey.sum(axis=1, keepdims=True)).astype(np.float32)


def _build_program(bias_vecs):
    """Build the 8-core SPMD Bass program."""
    import concourse.bacc as bacc
    import concourse.bass as bass
    import concourse.mybir as mybir
    import concourse.tile as tile

    f32 = mybir.dt.float32
    bf16 = mybir.dt.bfloat16
    i32 = mybir.dt.int32
    AL = mybir.AluOpType
    ACT = mybir.ActivationFunctionType

    nc = bacc.Bacc()

    # ---- DRAM tensors ----
    emb_d = nc.dram_tensor("emb", [V, E], bf16, kind="ExternalInput")
    idx_d = [nc.dram_tensor(f"idx{s}", [128, 128], i32, kind="ExternalInput")
             for s in range(2)]
    wct_d = nc.dram_tensor("wct", [128, 128], bf16, kind="ExternalInput")
    # gate slices g=0..5: (f,r) (f,z-neg) (b,r) (b,z-neg) (f,n) (b,n)
    wih_d = nc.dram_tensor("wih", [128, 6 * 128], bf16, kind="ExternalInput")
    whh_d = nc.dram_tensor("whh", [128, 6 * 128], bf16, kind="ExternalInput")
    fcw_d = nc.dram_tensor("fcw", [128, 4 * 2], bf16, kind="ExternalInput")
    bcm_d = nc.dram_tensor("bcm", [128, K], f32, kind="ExternalInput")
    out_d = nc.dram_tensor("out", [BL, 2], f32, kind="ExternalOutput")

    with tile.TileContext(nc) as tc:
        with tc.tile_pool(name="const", bufs=1) as cpool, \
             tc.tile_pool(name="gpool", bufs=8) as g_pool, \
             tc.tile_pool(name="xp", bufs=2) as x_pool, \
             tc.tile_pool(name="hkp", bufs=2) as hk_pool, \
             tc.tile_pool(name="encp", bufs=2) as enc_pool, \
             tc.tile_pool(name="gip", bufs=2) as gi_pool, \
             tc.tile_pool(name="grup", bufs=2) as gru_pool, \
             tc.tile_pool(name="stepp", bufs=12) as step_pool, \
             tc.tile_pool(name="psum_big", bufs=2, space="PSUM") as ps_big, \
             tc.tile_pool(name="psum_gru", bufs=2, space="PSUM") as ps_gru, \
             tc.tile_pool(name="dram", bufs=2, space="DRAM") as dram_pool:

            # ---- constants ----
            idx_t = []
            for s in range(2):
                t = cpool.tile([128, 128], i32, name=f"idx{s}_t")
                nc.sync.dma_start(t[:], idx_d[s][:])
                idx_t.append(t)
            wct = cpool.tile([128, 128], bf16, name="wct_t")
            nc.sync.dma_start(wct[:], wct_d[:])
            wih = cpool.tile([128, 6 * 128], bf16, name="wih_t")
            nc.sync.dma_start(wih[:], wih_d[:])
            whh = cpool.tile([128, 6 * 128], bf16, name="whh_t")
            nc.sync.dma_start(whh[:], whh_d[:])
            fcw = cpool.tile([128, 8], bf16, name="fcw_t")
            nc.sync.dma_start(fcw[:], fcw_d[:])
            bcm = cpool.tile([128, K], f32, name="bcm_t")
            nc.sync.dma_start(bcm[:], bcm_d[:])

            # ---- stmt-eighth-chunked phase A, outer-in pair delivery ----
            # flat gather index = g*2048 + k*128 + t_in*8 + prog (t_in 0..15).
            # Pair (g, 7-g) delivers fwd stmts 16g.. AND bwd stmts 112-16g..
            # -> GRU steps 16j..16j+16 run while pair j+1 is still gathering.
            encs = []   # per side enc [128, S*BL] bf16 (col = t*8 + prog)
            gis = []    # per side GI [128, S, 16] bf16 (nf | nb, bwd reversed)
            Xs, gstages = [], []
            for s in range(2):
                X = x_pool.tile([128, NT], bf16, tag="X", name=f"X{s}")
                Xs.append(X)
                gstages.append(dram_pool.tile([NT, 128], bf16, tag="gstage",
                                              name=f"gstage{s}"))
                encs.append(enc_pool.tile([128, NTREE], bf16, tag="enc",
                                          name=f"enc{s}"))
                gis.append(gi_pool.tile([128, S, 16], bf16, tag="GI",
                                        name=f"GI{s}"))

            QC = 2048   # flat rows / X cols per eighth-group

            def prep_grp(s, gq):
                X = Xs[s]
                # tree child-sum in raw-emb space (linear; W_c after)
                Xq = X[:, gq * QC:(gq + 1) * QC].rearrange(
                    "p (k n) -> p k n", k=K)
                for (pnode, cnode) in _EDGES:
                    nc.vector.tensor_tensor(
                        out=Xq[:, pnode, :], in0=Xq[:, pnode, :],
                        in1=Xq[:, cnode, :], op=AL.add)
                # W_c matmul; ACT relu-copy -> Hk (b_c==0 on the fast path,
                # so no per-k bias is needed here)
                Hk = hk_pool.tile([128, K, 128], bf16, tag="Hk")
                for j in range(4):              # 4 k's per free-512 matmul
                    ps = ps_big.tile([128, 512], f32, tag="wc")
                    nc.tensor.matmul(
                        ps[:], wct[:],
                        X[:, gq * QC + j * 512:gq * QC + (j + 1) * 512],
                        start=True, stop=True)
                    nc.scalar.activation(
                        Hk[:, 4 * j:4 * (j + 1), :],
                        ps.rearrange("p (k n) -> p k n", k=4)[:],
                        ACT.Relu)
                # tree-max over k (vector halving passes)
                half = K // 2
                while half > 1:
                    nc.vector.tensor_tensor(
                        out=Hk[:, 0:half, :], in0=Hk[:, 0:half, :],
                        in1=Hk[:, half:2 * half, :], op=AL.max)
                    half //= 2
                nc.vector.tensor_tensor(
                    out=encs[s][:, gq * 128:(gq + 1) * 128],
                    in0=Hk[:, 0, :], in1=Hk[:, 1, :], op=AL.max)
                # n-gate input matmuls for this group -> GI slots
                for d in range(2):              # 0=f (g=4), 1=b (g=5)
                    g = 4 + d
                    psg = ps_big.tile([128, 512], f32, tag="wc")
                    ps = psg[:, 0:128]
                    nc.tensor.matmul(
                        ps, wih[:, g * 128:(g + 1) * 128],
                        encs[s][:, gq * 128:(gq + 1) * 128],
                        start=True, stop=True)
                    psv = ps.rearrange("p (t b) -> p t b", t=16)
                    if d == 0:
                        dst = gis[s][:, gq * 16:(gq + 1) * 16, 0:BL]
                    else:
                        dst = gis[s][:, (7 - gq) * 16:(8 - gq) * 16, BL:2 * BL]
                        dst = dst[:, ::-1, :]
                    nc.scalar.copy(dst, psv[:])

            def gather_grp(s, gq):
                G = g_pool.tile([128, 16, 128], bf16, tag="G")
                for c in range(16):             # call = one k
                    gc = gq * 16 + c
                    nc.gpsimd.indirect_dma_start(
                        out=G[:, c, :], out_offset=None, in_=emb_d[:],
                        in_offset=bass.IndirectOffsetOnAxis(
                            ap=idx_t[s][:, gc:gc + 1], axis=0))
                gstage = gstages[s]
                nc.sync.dma_start(
                    gstage[gq * QC:(gq + 1) * QC, :]
                    .rearrange("(j p) e -> p j e", p=128),
                    G[:])
                nc.sync.dma_start_transpose(
                    Xs[s][:, gq * QC:(gq + 1) * QC],
                    gstage[gq * QC:(gq + 1) * QC, :])

            # ---- GRU: two independent per-side chains ----
            # H_s history [128, (S+1)*16] bf16; slot t cols: [f(8) | b(8)]
            hist = []
            for s in range(2):
                Hb = gru_pool.tile([128, (S + 1) * 16], bf16, name=f"H{s}",
                                   tag=f"H{s}", bufs=1)
                nc.vector.memset(Hb[:], 0)
                hist.append(Hb.rearrange("p (t b) -> p t b", b=16))

            # per-side per-step PSUM [128, 48]: rf rb zf zb nf nb (8 cols ea)
            # Each side is an independent dependency chain; emission order is
            # the intended steady-state time order (sides offset ~half step)
            # so strict engine FIFOs never head-of-line block across sides.
            # No GpSimd ops here: that engine runs the gather stream, which
            # overlaps the first GRU segment.
            def gru_step(t):
                for s in range(2):
                    tf, tb = t, S - 1 - t
                    hprev = hist[s][:, t, :]
                    encv = encs[s].rearrange("p (t b) -> p t b", t=S)
                    ps = ps_gru.tile([128, 48], f32, tag=f"gru{s}")
                    # (col, whh gate, h cols, wih gate or None, enc time)
                    mm = [(0, 0, 0, 0, tf), (8, 2, 8, 2, tb),
                          (16, 1, 0, 1, tf), (24, 3, 8, 3, tb),
                          (32, 4, 0, None, None), (40, 5, 8, None, None)]
                    for col, g, hc, gi_g, tg in mm:
                        nc.tensor.matmul(
                            ps[:, col:col + 8], whh[:, g * 128:(g + 1) * 128],
                            hprev[:, hc:hc + 8], start=True, stop=(gi_g is None))
                        if gi_g is not None:
                            nc.tensor.matmul(
                                ps[:, col:col + 8],
                                wih[:, gi_g * 128:(gi_g + 1) * 128],
                                encv[:, tg, :], start=False, stop=True)
                    # sigmoid over [rf rb zf zb] (z weights negated -> 1-z).
                    # Full 128 partitions everywhere: dead rows stay finite
                    # (PSUM pad rows are zeros from the zero-padded weights).
                    rz = step_pool.tile([128, 32], bf16, tag=f"rz{s}")
                    nc.scalar.activation(rz[:, :], ps[:, 0:32], ACT.Sigmoid)
                    m_ = step_pool.tile([128, 16], bf16, tag=f"m{s}")
                    nc.vector.tensor_tensor(out=m_[:, :], in0=ps[:, 32:48],
                                            in1=rz[:, 0:16], op=AL.mult)
                    av = step_pool.tile([128, 16], bf16, tag=f"av{s}")
                    nc.vector.tensor_tensor(out=av[:, :], in0=m_[:, :],
                                            in1=gis[s][:, t, :], op=AL.add)
                    # u/w fill the vector engine's tanh wait; post-tanh tail
                    # is then just v = z*nt and h' = w + v.
                    u_ = step_pool.tile([128, 16], bf16, tag=f"u{s}")
                    nc.vector.tensor_tensor(out=u_[:, :], in0=rz[:, 16:32],
                                            in1=hprev, op=AL.mult)
                    w_ = step_pool.tile([128, 16], bf16, tag=f"w{s}")
                    nc.vector.tensor_tensor(out=w_[:, :], in0=hprev,
                                            in1=u_[:, :], op=AL.subtract)
                    nt_ = step_pool.tile([128, 16], bf16, tag=f"nt{s}")
                    nc.scalar.activation(nt_[:, :], av[:, :], ACT.Tanh)
                    v_ = step_pool.tile([128, 16], bf16, tag=f"v{s}")
                    nc.vector.tensor_tensor(out=v_[:, :], in0=rz[:, 16:32],
                                            in1=nt_[:, :], op=AL.mult)
                    nc.vector.tensor_tensor(out=hist[s][:, t + 1, :],
                                            in0=w_[:, :], in1=v_[:, :],
                                            op=AL.add)

            # ---- emission: gather pair -> prep pair -> GRU segment ----
            for pi in range(4):
                ga, gb = pi, 7 - pi
                for s in range(2):
                    for q in (ga, gb):
                        gather_grp(s, q)
                for s in range(2):
                    for q in (ga, gb):
                        prep_grp(s, q)
                for t in range(16 * pi, 16 * (pi + 1)):
                    gru_step(t)
            for t in range(64, S):
                gru_step(t)

            # ---- max-pool over time: tree reduction on the h history ----
            pooled = []
            for s in range(2):
                Hv = hist[s]
                n = S
                base = 1
                while n > 1:
                    half = n // 2
                    nc.vector.tensor_tensor(
                        out=Hv[0:100, base:base + half, :],
                        in0=Hv[0:100, base:base + half, :],
                        in1=Hv[0:100, base + half:base + 2 * half, :], op=AL.max)
                    n = half
                pooled.append(Hv[:, base, :])

            # ---- fc + softmax ----
            # vec chunks: 0=fwd_L 1=bwd_L 2=fwd_R 3=bwd_R; pooled[s]: 0:8 f, 8:16 b
            psf = ps_gru.tile([128, 8], f32, tag="fc", bufs=1)
            chunks = [(0, 0), (0, 8), (1, 0), (1, 8)]
            for ci, (sd, col) in enumerate(chunks):
                nc.tensor.matmul(
                    psf[0:2, :], fcw[:, ci * 2:(ci + 1) * 2],
                    pooled[sd][:, col:col + 8],
                    start=(ci == 0), stop=(ci == 3))
            t32 = step_pool.tile([128, 32], f32, tag="t32")
            nc.vector.memset(t32[0:32, :], 0)
            nc.vector.tensor_copy(t32[0:2, 0:8], psf[0:2, :])
            t32b = step_pool.tile([128, 32], f32, tag="t32b")
            nc.vector.transpose(t32b[0:32, :], t32[0:32, :])
            dcol = step_pool.tile([128, 2], f32, tag="dcol")
            nc.vector.tensor_tensor(out=dcol[0:8, 0:1], in0=t32b[0:8, 0:1],
                                    in1=t32b[0:8, 1:2], op=AL.subtract)
            outt = step_pool.tile([128, 2], f32, tag="outt")
            nc.scalar.activation(outt[0:8, 0:1], dcol[0:8, 0:1], ACT.Sigmoid,
                                 bias=float(bias_vecs["fc_db"]))
            nc.vector.tensor_scalar(
                out=outt[0:8, 1:2], in0=outt[0:8, 0:1], scalar1=-1.0, scalar2=1.0,
                op0=AL.mult, op1=AL.add)
            nc.sync.dma_start(out_d[:], outt[0:8, 0:2])

    nc.compile()
    return nc


_CACHED = {}


def kernel(**inputs):
    inputs = {k: np.asarray(v) for k, v in inputs.items()}
    tokens1 = inputs["tokens1"].astype(np.int64)
    tokens2 = inputs["tokens2"].astype(np.int64)
    parent = inputs["parent"].astype(np.int64)
    level = inputs["level"].astype(np.int64)
    emb = inputs["emb"].astype(np.float32)
    W_c = inputs["W_c"].astype(np.float32)
    b_c = inputs["b_c"].astype(np.float32)
    fc_w = inputs["fc_w"].astype(np.float32)
    fc_b = inputs["fc_b"].astype(np.float32)
    gw = {k: inputs[k].astype(np.float32) for k in (
        "gru_wih_f", "gru_whh_f", "gru_bih_f", "gru_bhh_f",
        "gru_wih_b", "gru_whh_b", "gru_bih_b", "gru_bhh_b")}

    # verify the fixed tree topology this kernel specializes for
    base = np.arange(B * S, dtype=np.int64)[:, None] * K
    exp_parent = (base + _LOCAL_PARENT[None, :]).reshape(-1)
    exp_level = np.tile(_LOCAL_LEVEL, B * S)
    zero_bias = not b_c.any() and all(
        not gw[k].any() for k in ("gru_bih_f", "gru_bhh_f", "gru_bih_b", "gru_bhh_b"))
    if not (np.array_equal(parent, exp_parent) and np.array_equal(level, exp_level)
            and zero_bias):
        return _np_reference(tokens1, tokens2, parent, level, emb, W_c, b_c,
                             gw["gru_wih_f"], gw["gru_whh_f"], gw["gru_bih_f"],
                             gw["gru_bhh_f"], gw["gru_wih_b"], gw["gru_whh_b"],
                             gw["gru_bih_b"], gw["gru_bhh_b"], fc_w, fc_b)

    # ---- host-side weight packing (layout prep only) ----
    bf = ml_dtypes.bfloat16
    emb16 = emb.astype(bf)                                         # [V, 128]
    wct = np.ascontiguousarray(W_c.T).astype(bf)                   # [128,128] lhsT
    # gate order: 0=(f,r) 1=(f,z) 2=(b,r) 3=(b,z) 4=(f,n) 5=(b,n); z negated
    def pack_w(w, negate):  # w [100, D] -> [D, 128] lhsT padded
        out = np.zeros((w.shape[1], 128), np.float32)
        out[:, :100] = w.T * (-1.0 if negate else 1.0)
        return out
    gates = [("f", 0, False), ("f", 1, True), ("b", 0, False),
             ("b", 1, True), ("f", 2, False), ("b", 2, False)]
    wih = np.concatenate(
        [pack_w(gw[f"gru_wih_{d}"][gi * H:(gi + 1) * H], neg)
         for d, gi, neg in gates], axis=1).astype(bf)               # [128, 6*128]
    whh_full = np.concatenate(
        [pack_w(gw[f"gru_whh_{d}"][gi * H:(gi + 1) * H], neg)
         for d, gi, neg in gates], axis=1)                          # [100, 6*128]
    whh = np.zeros((128, 6 * 128), np.float32)
    whh[:H] = whh_full
    whh = whh.astype(bf)
    fcw = np.zeros((128, 8), np.float32)
    for ci in range(4):                                            # chunks of 100
        fcw[:H, ci * 2:(ci + 1) * 2] = fc_w[:, ci * H:(ci + 1) * H].T
    fcw = fcw.astype(bf)
    bcm = np.zeros((128, K), np.float32)
    bcm[:E] = b_c[:, None] * _SUBSIZE[None, :]
    bias_vecs = {"fc_db": float(fc_b[0] - fc_b[1])}

    # ---- per-core gather index arrays ----
    # flat gather order: flat = g*2048 + k*128 + t_in*8 + prog (stmt=16g+t_in)
    def idx_for(tokens, core):
        t3 = tokens.reshape(B, S, K)[core * BL:(core + 1) * BL]    # [8,128,16]
        t4 = t3.reshape(BL, 8, 16, K)                              # p,g,ti,k
        flat = np.transpose(t4, (1, 3, 2, 0)).reshape(-1)          # g,k,ti,p
        return flat.reshape(128, 128).T.astype(np.int32).copy()    # [p, call]

    from concourse.bass_utils import run_bass_kernel_spmd

    key = ("prog_v2", bias_vecs["fc_db"])
    if key not in _CACHED:
        _CACHED[key] = _build_program(bias_vecs)
    nc = _CACHED[key]

    in_maps = []
    for c in range(NCORES):
        in_maps.append({
            "emb": emb16,
            "idx0": idx_for(tokens1, c),
            "idx1": idx_for(tokens2, c),
            "wct": wct, "wih": wih, "whh": whh, "fcw": fcw,
            "bcm": bcm,
        })

    if os.environ.get("BPCC_SIM"):
        from concourse.bass_interp import CoreSim
        sim = CoreSim(nc)
        for k, v in in_maps[0].items():
            sim.tensor(k)[:] = v
        sim.simulate()
        o0 = np.asarray(sim.tensor("out")).copy()
        return np.vstack([o0] * NCORES).astype(np.float32)

    trace = bool(os.environ.get("BPCC_TRACE"))
    res = run_bass_kernel_spmd(nc, in_maps, core_ids=list(range(NCORES)),
                               trace=trace,
                               tmpdir=os.environ.get("BPCC_TRACE_DIR") or None)
    if trace and res.exec_time_ns:
        print(f"HW exec time: {res.exec_time_ns} ns")
    out = np.vstack([res.results[c]["out"] for c in range(NCORES)])
    return out.astype(np.float32)
